# revision 1
# baseline (speedup 1.0000x reference)
"""Kalman CV filter (nn_KalmanCV) — Trainium2 Bass kernel, 8-core data parallel.

Math: the covariance P (and thus the Kalman gains K_t and the output
channels sx/sy/rho) is batch-independent — it depends only on the scalar
inputs. The whole per-batch computation therefore collapses to a linear
map over the 32 history scalars:

    out[l, b, ch<2] = sum_{t,ci} W[t*2+ci, l*5+ch] * hist[t, b, ci]
    out[l, b, ch>=2] = const[l, ch]          (sx, sy, rho)

Device kernel per core: tiled matmul (32x125)^T @ (32x512) on the PE with
a per-partition bias add (the constant channels) on the ScalarE, then DMA
out. Batch is sharded across 8 cores; layout transforms happen host-side.
"""
import numpy as np

DT = 0.2
LEN_HIST = 16
LEN_PRED = 25
BATCH = 100000

N_CORES = 8
NB = 512                    # batch columns per matmul tile
NTILES = 25                 # tiles per core
BS = NB * NTILES            # 12800 padded batch per core
BS_REAL = BATCH // N_CORES  # 12500
P_OUT = 5 * LEN_PRED        # 125
K_IN = 2 * LEN_HIST         # 32


def _build_wc(vsx, vsy, asx, asy, GR, coef_G, len_pred):
    """Collapse the filter to W (32, 5L) and constant vector cvec (5L,)."""
    L = int(len_pred)
    H = np.zeros((2, 4)); H[0, 0] = 1.0; H[1, 2] = 1.0
    F = np.eye(4); F[0, 1] = DT; F[2, 3] = DT
    G = np.array([DT * DT / 2, DT, DT * DT / 2, DT])
    Id = np.eye(4)

    ax2 = float(asx[0]) ** 2
    ay2 = float(asy[0]) ** 2
    mx = np.array([1.0, 1.0, 0.0, 0.0]); my = 1.0 - mx
    scale = (ax2 * np.outer(mx, mx) + ay2 * np.outer(my, my)
             + np.outer(mx, my) + np.outer(my, mx))
    g = G * np.tanh(np.asarray(coef_G, np.float64))
    Q = np.outer(g, g) * scale
    R = np.outer(np.asarray(GR, np.float64), np.asarray(GR, np.float64))

    D0 = np.array([[1.0, 0.0], [-1.0 / DT, 0.0], [0.0, 1.0], [0.0, -1.0 / DT]])
    D1 = np.array([[0.0, 0.0], [1.0 / DT, 0.0], [0.0, 0.0], [0.0, 1.0 / DT]])
    P = np.diag([R[0, 0], float(vsx[0]) ** 2, R[1, 1], float(vsy[0]) ** 2])

    C = np.zeros((LEN_HIST, 4, 2))
    C[0] = D0; C[1] = D1
    for t in range(1, LEN_HIST):
        P = F @ P @ F.T + Q
        S = H @ P @ H.T + R
        K = P @ H.T @ np.linalg.inv(S)
        A = (Id - K @ H) @ F
        C = np.einsum('ij,tjk->tik', A, C)
        C[t] += K
        ImKH = Id - K @ H
        P = ImKH @ P @ ImKH.T + K @ R @ K.T

    W_dev = np.zeros((K_IN, 5 * L))
    cvec = np.zeros(5 * L)
    M = np.eye(4)
    for l in range(L):
        M = F @ M
        P = F @ P @ F.T + Q
        HFl = H @ M
        Wl = np.einsum('ij,tjk->itk', HFl, C)   # (2, T, 2)
        for ch in range(2):
            W_dev[:, l * 5 + ch] = Wl[ch].reshape(-1)
        Pout = H @ P @ H.T
        sx = np.sqrt(Pout[0, 0]); sy = np.sqrt(Pout[1, 1])
        cvec[l * 5 + 2] = sx
        cvec[l * 5 + 3] = sy
        cvec[l * 5 + 4] = (Pout[0, 1] + Pout[1, 0]) / (2.0 * sx * sy)
    return W_dev.astype(np.float32), cvec.astype(np.float32)


_NC_CACHE = {}


def _build_bass():
    import concourse.bass as bass
    import concourse.bacc as bacc
    import concourse.tile as tile
    from concourse import mybir

    nc = bacc.Bacc("TRN2", target_bir_lowering=False, debug=False,
                   num_devices=N_CORES)
    x = nc.declare_dram_parameter("x", [K_IN, BS], mybir.dt.float32, isOutput=False)
    w = nc.declare_dram_parameter("w", [K_IN, P_OUT], mybir.dt.float32, isOutput=False)
    cv = nc.declare_dram_parameter("cv", [P_OUT, 1], mybir.dt.float32, isOutput=False)
    out = nc.declare_dram_parameter("out", [P_OUT, BS], mybir.dt.float32, isOutput=True)

    with tile.TileContext(nc) as tc:
        with tc.tile_pool(name="singles", bufs=1) as singles, \
             tc.tile_pool(name="xin", bufs=4) as xin_pool, \
             tc.tile_pool(name="ps", bufs=4, space="PSUM") as psum_pool, \
             tc.tile_pool(name="op", bufs=4) as out_pool:
            w_tile = singles.tile([K_IN, P_OUT], mybir.dt.float32)
            nc.sync.dma_start(out=w_tile, in_=w[:, :])
            c_tile = singles.tile([P_OUT, 1], mybir.dt.float32)
            nc.sync.dma_start(out=c_tile, in_=cv[:, :])
            # Group 5 matmul tiles per DMA: 32x2560 in (320KB), 125x2560
            # out (1.25MB) — above the DMA efficiency knee.
            GRP = 5
            for g in range(NTILES // GRP):
                x_tile = xin_pool.tile([K_IN, GRP * NB], mybir.dt.float32)
                nc.sync.dma_start(
                    out=x_tile, in_=x[:, g * GRP * NB:(g + 1) * GRP * NB])
                o_tile = out_pool.tile([P_OUT, GRP * NB], mybir.dt.float32)
                for j in range(GRP):
                    ps = psum_pool.tile([P_OUT, NB], mybir.dt.float32)
                    nc.tensor.matmul(ps, w_tile,
                                     x_tile[:, j * NB:(j + 1) * NB],
                                     start=True, stop=True)
                    nc.scalar.activation(
                        out=o_tile[:, j * NB:(j + 1) * NB], in_=ps,
                        func=mybir.ActivationFunctionType.Identity,
                        bias=c_tile, scale=1.0,
                    )
                nc.sync.dma_start(
                    out=out[:, g * GRP * NB:(g + 1) * GRP * NB], in_=o_tile)
    nc.compile()
    return nc


def _get_nc():
    if "nc" not in _NC_CACHE:
        _NC_CACHE["nc"] = _build_bass()
    return _NC_CACHE["nc"]


def _run_device(hist_T, W, cvec, trace=False):
    from concourse.bass_utils import run_bass_kernel_spmd

    cv2 = cvec.reshape(P_OUT, 1)
    in_maps = []
    for c in range(N_CORES):
        shard = np.zeros((K_IN, BS), np.float32)
        shard[:, :BS_REAL] = hist_T[:, c * BS_REAL:(c + 1) * BS_REAL]
        in_maps.append({"x": shard, "w": W, "cv": cv2})
    res = run_bass_kernel_spmd(_get_nc(), in_maps, list(range(N_CORES)),
                               trace=trace)
    return res


def kernel(hist, velocity_std_x, velocity_std_y, acceleration_std_x,
           acceleration_std_y, GR, coef_G, len_pred):
    hist = np.asarray(hist, np.float32)
    L = int(len_pred)
    W, cvec = _build_wc(velocity_std_x, velocity_std_y, acceleration_std_x,
                        acceleration_std_y, GR, coef_G, L)
    T, B, _ = hist.shape
    hist_T = np.ascontiguousarray(hist.transpose(0, 2, 1)).reshape(2 * T, B)

    if L != LEN_PRED or B != BATCH or T != LEN_HIST:
        # shape surprise: fall back to exact host math
        out_flat = W.T @ hist_T + cvec[:, None]
        return np.ascontiguousarray(
            out_flat.reshape(L, 5, B).transpose(0, 2, 1)).astype(np.float32)

    res = _run_device(hist_T, W, cvec)
    out = np.empty((LEN_PRED, B, 5), np.float32)
    for c in range(N_CORES):
        oc = res.results[c]["out"][:, :BS_REAL]          # (125, 12500)
        out[:, c * BS_REAL:(c + 1) * BS_REAL, :] = (
            oc.reshape(LEN_PRED, 5, BS_REAL).transpose(0, 2, 1))
    return out



# revision 2
# speedup vs baseline: 3.4085x; 3.4085x over previous
"""Kalman CV filter (nn_KalmanCV) — Trainium2 Bass kernel, 8-core data parallel.

Math: the covariance P (and thus the Kalman gains K_t and the output
channels sx/sy/rho) is batch-independent — it depends only on the scalar
inputs. The whole per-batch computation therefore collapses to a linear
map over the 32 history scalars:

    out[l, b, ch<2] = sum_{t,ci} W[t*2+ci, l*2+ch] * hist[t, b, ci]
    out[l, b, ch>=2] = const[l, ch]          (sx, sy, rho)

Only the 50 data-dependent mu rows are computed on device; the 75
constant rows are filled host-side (they are input-data-independent,
like W itself). The matmul runs in fp16 (tolerance 2e-2, fp16 path
measures ~5e-4): 8x the fp32 PE rate and half the HBM bytes.

To double engine efficiency, two batch half-shards are packed into one
matmul with block-diagonal weights: lhsT (64, 100) with W in blocks
[0:32, 0:50] and [32:64, 50:100], rhs (64, n) holding half A in
partitions 0-31 and half B in 32-63. PSUM (100, n) then carries both
halves, so the PSUM->SBUF cast-copies (alternating Vector/Scalar
engines) run at 100/128 partition utilization instead of 50/128.
"""
import numpy as np

DT = 0.2
LEN_HIST = 16
LEN_PRED = 25
BATCH = 100000

N_CORES = 8
BS = 12800                  # padded batch per core
BS_REAL = BATCH // N_CORES  # 12500
HB = BS // 2                # 6400 columns per half-shard
K2 = 64                     # 2 stacked blocks of 32 history scalars
M2 = 100                    # 2 stacked blocks of 50 mu rows
NB = 512                    # matmul tile width (one PSUM bank)
# DMA groups over the 6400 columns: 5+5+3 matmul tiles
GROUPS = [(0, 2560), (2560, 2560), (5120, 1280)]


def _build_wc(vsx, vsy, asx, asy, GR, coef_G, len_pred):
    """Collapse the filter to W (32, 5L) and constant vector cvec (5L,)."""
    L = int(len_pred)
    H = np.zeros((2, 4)); H[0, 0] = 1.0; H[1, 2] = 1.0
    F = np.eye(4); F[0, 1] = DT; F[2, 3] = DT
    G = np.array([DT * DT / 2, DT, DT * DT / 2, DT])
    Id = np.eye(4)

    ax2 = float(asx[0]) ** 2
    ay2 = float(asy[0]) ** 2
    mx = np.array([1.0, 1.0, 0.0, 0.0]); my = 1.0 - mx
    scale = (ax2 * np.outer(mx, mx) + ay2 * np.outer(my, my)
             + np.outer(mx, my) + np.outer(my, mx))
    g = G * np.tanh(np.asarray(coef_G, np.float64))
    Q = np.outer(g, g) * scale
    R = np.outer(np.asarray(GR, np.float64), np.asarray(GR, np.float64))

    D0 = np.array([[1.0, 0.0], [-1.0 / DT, 0.0], [0.0, 1.0], [0.0, -1.0 / DT]])
    D1 = np.array([[0.0, 0.0], [1.0 / DT, 0.0], [0.0, 0.0], [0.0, 1.0 / DT]])
    P = np.diag([R[0, 0], float(vsx[0]) ** 2, R[1, 1], float(vsy[0]) ** 2])

    C = np.zeros((LEN_HIST, 4, 2))
    C[0] = D0; C[1] = D1
    for t in range(1, LEN_HIST):
        P = F @ P @ F.T + Q
        S = H @ P @ H.T + R
        K = P @ H.T @ np.linalg.inv(S)
        A = (Id - K @ H) @ F
        C = np.einsum('ij,tjk->tik', A, C)
        C[t] += K
        ImKH = Id - K @ H
        P = ImKH @ P @ ImKH.T + K @ R @ K.T

    W_dev = np.zeros((2 * LEN_HIST, 5 * L))
    cvec = np.zeros(5 * L)
    M = np.eye(4)
    for l in range(L):
        M = F @ M
        P = F @ P @ F.T + Q
        HFl = H @ M
        Wl = np.einsum('ij,tjk->itk', HFl, C)   # (2, T, 2)
        for ch in range(2):
            W_dev[:, l * 5 + ch] = Wl[ch].reshape(-1)
        Pout = H @ P @ H.T
        sx = np.sqrt(Pout[0, 0]); sy = np.sqrt(Pout[1, 1])
        cvec[l * 5 + 2] = sx
        cvec[l * 5 + 3] = sy
        cvec[l * 5 + 4] = (Pout[0, 1] + Pout[1, 0]) / (2.0 * sx * sy)
    return W_dev.astype(np.float32), cvec.astype(np.float32)


_NC_CACHE = {}


def _build_bass():
    import concourse.bass as bass
    import concourse.bacc as bacc
    import concourse.tile as tile
    from concourse import mybir

    nc = bacc.Bacc("TRN2", target_bir_lowering=False, debug=False,
                   num_devices=N_CORES)
    x = nc.declare_dram_parameter("x", [K2, HB], mybir.dt.float16, isOutput=False)
    w = nc.declare_dram_parameter("w", [K2, M2], mybir.dt.float16, isOutput=False)
    out = nc.declare_dram_parameter("out", [M2, HB], mybir.dt.float16, isOutput=True)

    with tile.TileContext(nc) as tc:
        with tc.tile_pool(name="singles", bufs=1) as singles, \
             tc.tile_pool(name="xin", bufs=3) as xin_pool, \
             tc.tile_pool(name="ps", bufs=4, space="PSUM") as psum_pool, \
             tc.tile_pool(name="op", bufs=3) as out_pool:
            w_tile = singles.tile([K2, M2], mybir.dt.float16)
            nc.sync.dma_start(out=w_tile, in_=w[:, :])
            mm = 0
            for goff, gw in GROUPS:
                x_tile = xin_pool.tile([K2, gw], mybir.dt.float16)
                nc.sync.dma_start(out=x_tile, in_=x[:, goff:goff + gw])
                o_tile = out_pool.tile([M2, gw], mybir.dt.float16)
                off = 0
                while off < gw:
                    nw = min(NB, gw - off)
                    ps = psum_pool.tile([M2, nw], mybir.dt.float32)
                    nc.tensor.matmul(ps, w_tile, x_tile[:, off:off + nw],
                                     start=True, stop=True)
                    dst = o_tile[:, off:off + nw]
                    if mm % 2 == 0:
                        nc.vector.tensor_copy(out=dst, in_=ps)
                    else:
                        nc.scalar.copy(out=dst, in_=ps)
                    mm += 1
                    off += nw
                nc.sync.dma_start(out=out[:, goff:goff + gw], in_=o_tile)
    nc.compile()
    return nc


def _get_nc():
    if "nc" not in _NC_CACHE:
        _NC_CACHE["nc"] = _build_bass()
    return _NC_CACHE["nc"]


def _run_device(x_shards, W2, trace=False):
    from concourse.bass_utils import run_bass_kernel_spmd

    in_maps = [{"x": shard, "w": W2} for shard in x_shards]
    return run_bass_kernel_spmd(_get_nc(), in_maps, list(range(N_CORES)),
                                trace=trace)


def _make_shards(hist_T16):
    """hist_T16: (32, BATCH) f16 -> per-core (64, HB) stacked half-shards."""
    shards = []
    for c in range(N_CORES):
        xc = hist_T16[:, c * BS_REAL:(c + 1) * BS_REAL]  # (32, 12500)
        shard = np.zeros((K2, HB), np.float16)
        shard[0:32, :] = xc[:, :HB]
        shard[32:64, :BS_REAL - HB] = xc[:, HB:]
        shards.append(shard)
    return shards


def kernel(hist, velocity_std_x, velocity_std_y, acceleration_std_x,
           acceleration_std_y, GR, coef_G, len_pred):
    hist = np.asarray(hist, np.float32)
    L = int(len_pred)
    W, cvec = _build_wc(velocity_std_x, velocity_std_y, acceleration_std_x,
                        acceleration_std_y, GR, coef_G, L)
    T, B, _ = hist.shape
    hist_T = np.ascontiguousarray(hist.transpose(0, 2, 1)).reshape(2 * T, B)

    if L != LEN_PRED or B != BATCH or T != LEN_HIST:
        # shape surprise: fall back to exact host math
        out_flat = W.T @ hist_T + cvec[:, None]
        return np.ascontiguousarray(
            out_flat.reshape(L, 5, B).transpose(0, 2, 1)).astype(np.float32)

    # mu-only weight (32, 50), stacked block-diagonally to (64, 100)
    mu_cols = np.array([l * 5 + ch for l in range(LEN_PRED) for ch in range(2)])
    W_mu = W[:, mu_cols].astype(np.float16)
    W2 = np.zeros((K2, M2), np.float16)
    W2[0:32, 0:50] = W_mu
    W2[32:64, 50:100] = W_mu

    res = _run_device(_make_shards(hist_T.astype(np.float16)), W2)

    out = np.empty((LEN_PRED, B, 5), np.float32)
    consts = cvec.reshape(LEN_PRED, 5)[:, 2:5]           # (25, 3)
    out[:, :, 2:5] = consts[:, None, :]
    for c in range(N_CORES):
        oc = res.results[c]["out"]                       # (100, 6400) f16
        mu = np.concatenate(
            [oc[0:50, :], oc[50:100, :BS_REAL - HB]], axis=1)  # (50, 12500)
        out[:, c * BS_REAL:(c + 1) * BS_REAL, 0:2] = (
            mu.reshape(LEN_PRED, 2, BS_REAL).transpose(0, 2, 1))
    return out
